# revision 13
# baseline (speedup 1.0000x reference)
"""Causal BoW (running mean over T) Trainium2 kernel.

out[b, t, c] = sum_{s<=t} x[b, s, c] / (t+1)   for x of shape [32, 2048, 512] f32.

Sharding: batch B=32 across 8 NeuronCores (4 samples each), no cross-core comms.

Per-core algorithm (per sample [T=2048, C=512]):
  - T is tiled into 17 blocks of R=127 rows (16 full + a 16-row tail). Each
    SBUF block tile has 128 partitions: rows 0..126 hold x rows, row 127
    holds the block's running-prefix offset (the sum of all earlier blocks).
  - f32 matmuls cost 4 cycles/row on the PE; float32r costs ~1 cycle/row but
    keeps only 11 mantissa bits. x is split on-chip into
    x_hi = round_f32r(x) (ACT copy) and x_lo = round_f32r(x - x_hi) (DVE sub);
    streaming both through the PE reconstructs full fp32 precision (verified
    bit-exact on HW). The offset rows are split the same way (off_hi row goes
    into the x_hi tile, off_lo into the x_lo tile).
  - Offsets: 32 accumulating matmuls with "step" selector weights
    (step_k[p, m] = 1 if m > k, zero for the offset row p=127) produce
    off[m, c] = sum_{k<m} tot_k in one PSUM bank; split hi/lo and scattered
    into partition 127 of every block slot by one small SBUF->SBUF DMA per
    tile (DMA moves data across partitions freely; compute engines cannot).
  - Main scan: psum_j = U'^T.T @ xhi_j + U'^T.T @ xlo_j with a single shared
    weight matrix U' (U'[k, m] = 1 if k <= m < 127, U'[127, m] = 1: causal
    prefix within the block plus the offset row broadcast to every output
    row). One weight matrix for all 34 matmuls per sample keeps the PE
    streaming back-to-back at ~239 ns per N=512 matmul.
  - Eviction: Copy with per-partition scale recip[p, j] = 1/(j*127+p+1)
    applied while moving PSUM -> SBUF, alternating ACT/DVE.
"""

import numpy as np

import concourse.bass as bass
import concourse.bacc as bacc
import concourse.mybir as mybir
from concourse import tile
from concourse.bass_utils import run_bass_kernel_spmd

B, T, C = 32, 2048, 512
N_CORES = 8
BS = B // N_CORES          # samples per core
P = 128                    # partitions
R = P - 1                  # x rows per block (127)
NFULL = T // R             # 16 full blocks
TAIL = T - NFULL * R       # 16 rows in the tail block
NBLK = NFULL + 1           # 17 blocks
GA = 8                     # blocks 0..7 in tile group A
GB = NBLK - GA             # blocks 8..16 (incl. tail) in tile group B
F32 = mybir.dt.float32
F32R = mybir.dt.float32r

_cache = {}


def _build():
    nc = bacc.Bacc()
    x = nc.dram_tensor("x", [BS, T, C], F32, kind="ExternalInput")
    uprime = nc.dram_tensor("uprime", [P, P], F32R, kind="ExternalInput")
    stepm = nc.dram_tensor("stepm", [P, NFULL * NBLK], F32R, kind="ExternalInput")
    recip = nc.dram_tensor("recip", [P, NBLK], F32, kind="ExternalInput")
    y = nc.dram_tensor("y", [BS, T, C], F32, kind="ExternalOutput")

    with tile.TileContext(nc) as tc:
        with (
            tc.tile_pool(name="singles", bufs=1) as singles,
            tc.tile_pool(name="xp", bufs=2) as xpool,
            tc.tile_pool(name="xhp", bufs=2) as xhpool,
            tc.tile_pool(name="xlp", bufs=2) as xlpool,
            tc.tile_pool(name="op", bufs=2) as opool,
            tc.tile_pool(name="offp", bufs=2) as offpool,
            tc.tile_pool(name="pblk", bufs=5, space="PSUM") as pblk,
            tc.tile_pool(name="poff", bufs=2, space="PSUM") as poff,
        ):
            u_t = singles.tile([P, P], F32R)
            nc.sync.dma_start(out=u_t[:], in_=uprime[:])
            step_t = singles.tile([P, NFULL * NBLK], F32R)
            nc.sync.dma_start(out=step_t[:], in_=stepm[:])
            recip_t = singles.tile([P, NBLK], F32)
            nc.sync.dma_start(out=recip_t[:], in_=recip[:])

            for b in range(BS):
                # group A: blocks 0..7, group B: blocks 8..15 + tail
                xts, xhs, xls = [], [], []
                for g, (j0, nb) in enumerate(((0, GA), (GA, GB))):
                    nfull = min(nb, NFULL - j0)       # full 127-row blocks here
                    xt = xpool.tile([P, nb * C], F32, tag="xt")
                    xt3 = xt.rearrange("p (j c) -> p j c", c=C)
                    xs = x[b, j0 * R:j0 * R + nfull * R].rearrange(
                        "(j p) c -> p j c", p=R
                    )
                    # zero the offset row (96:128 covers it with a legal
                    # partition base) and the tail slot's unused rows before
                    # the DMAs land
                    nc.gpsimd.memset(xt[96:P, :], 0.0)
                    nc.sync.dma_start(out=xt3[:R, 0:nfull, :], in_=xs)
                    if nfull < nb:                    # tail rows
                        nc.gpsimd.memset(xt3[0:96, nb - 1, :], 0.0)
                        nc.sync.dma_start(
                            out=xt3[0:TAIL, nb - 1, :],
                            in_=x[b, NFULL * R:T],
                        )
                    xh = xhpool.tile([P, nb * C], F32R, tag="xh")
                    nc.scalar.copy(out=xh[:], in_=xt[:])
                    xl = xlpool.tile([P, nb * C], F32R, tag="xl")
                    nc.vector.tensor_sub(
                        out=xl[:], in0=xt[:], in1=xh[:].bitcast(F32)
                    )
                    xts.append(xt)
                    xhs.append(xh)
                    xls.append(xl)

                # off[m, c] = sum_{k<m} (block-k column sum), one PSUM bank
                offp_t = poff.tile([NBLK, C], F32)
                for k in range(NFULL):
                    sel = step_t[:, k * NBLK:(k + 1) * NBLK]
                    g, jj = (0, k) if k < GA else (1, k - GA)
                    for part, src in ((0, xhs), (1, xls)):
                        nc.tensor.matmul(
                            offp_t[:],
                            sel,
                            src[g][:, jj * C:(jj + 1) * C],
                            start=(k == 0 and part == 0),
                            stop=(k == NFULL - 1 and part == 1),
                        )
                off_hi = offpool.tile([NBLK, C], F32R, tag="offhi")
                nc.scalar.copy(out=off_hi[:], in_=offp_t[:])
                off_lo = offpool.tile([NBLK, C], F32R, tag="offlo")
                nc.vector.tensor_sub(
                    out=off_lo[:], in0=offp_t[:], in1=off_hi[:].bitcast(F32)
                )

                # scatter offset rows into partition 127 of each block slot
                for g, (j0, nb) in enumerate(((0, GA), (GA, GB))):
                    xh3 = xhs[g].rearrange("p (j c) -> p j c", c=C)
                    xl3 = xls[g].rearrange("p (j c) -> p j c", c=C)
                    nc.sync.dma_start(
                        out=xh3[R:P, 0:nb, :], in_=off_hi[j0:j0 + nb, :]
                    )
                    nc.sync.dma_start(
                        out=xl3[R:P, 0:nb, :], in_=off_lo[j0:j0 + nb, :]
                    )

                # main scan: all matmuls share the single weight matrix U'
                for g, (j0, nb) in enumerate(((0, GA), (GA, GB))):
                    ot = opool.tile([P, nb * C], F32, tag="ot")
                    for jj in range(nb):
                        j = j0 + jj
                        cs = slice(jj * C, (jj + 1) * C)
                        pb = pblk.tile([P, C], F32)
                        nc.tensor.matmul(pb[:], u_t[:], xhs[g][:, cs],
                                         start=True, stop=False)
                        nc.tensor.matmul(pb[:], u_t[:], xls[g][:, cs],
                                         start=False, stop=True)
                        if j % 2 == 0:
                            nc.scalar.mul(ot[:R, cs], pb[:R, :],
                                          recip_t[:R, j:j + 1])
                        else:
                            nc.vector.tensor_scalar_mul(
                                ot[:R, cs], pb[:R, :], recip_t[:R, j:j + 1]
                            )
                    ot3 = ot.rearrange("p (j c) -> p j c", c=C)
                    nfull = min(nb, NFULL - j0)
                    ys = y[b, j0 * R:j0 * R + nfull * R].rearrange(
                        "(j p) c -> p j c", p=R
                    )
                    nc.sync.dma_start(out=ys, in_=ot3[:R, 0:nfull, :])
                    if nfull < nb:
                        nc.sync.dma_start(
                            out=y[b, NFULL * R:T], in_=ot3[0:TAIL, nb - 1, :]
                        )
    nc.finalize()
    return nc


def _consts():
    # U'[k, m] = 1 if k <= m < 127 (causal within block), U'[127, m] = 1
    # (offset row broadcast); column 127 unused.
    u = np.zeros((P, P), dtype=np.float32)
    for k in range(R):
        u[k, k:R] = 1.0
    u[R, :R] = 1.0
    # step_k[p, m] = 1 if m > k, for x rows p<127 only
    step = np.zeros((P, NFULL * NBLK), dtype=np.float32)
    for k in range(NFULL):
        for m in range(NBLK):
            if m > k:
                step[:R, k * NBLK + m] = 1.0
    recip = np.ones((P, NBLK), dtype=np.float32)
    t = np.arange(1, T + 1, dtype=np.float32)
    for j in range(NBLK):
        n = R if j < NFULL else TAIL
        recip[:n, j] = 1.0 / t[j * R:j * R + n]
    return u, step, recip


def run(x, trace=False):
    x = np.ascontiguousarray(np.asarray(x, dtype=np.float32))
    assert x.shape == (B, T, C), x.shape
    if "nc" not in _cache:
        _cache["nc"] = _build()
    nc = _cache["nc"]
    u, step, recip = _consts()
    in_maps = [
        {
            "x": np.ascontiguousarray(x[i * BS:(i + 1) * BS]),
            "uprime": u,
            "stepm": step,
            "recip": recip,
        }
        for i in range(N_CORES)
    ]
    res = run_bass_kernel_spmd(nc, in_maps, list(range(N_CORES)), trace=trace)
    y = np.concatenate([res.results[i]["y"] for i in range(N_CORES)], axis=0)
    return y, res.exec_time_ns


def kernel(x):
    y, _ = run(x, trace=False)
    return y


# revision 14
# speedup vs baseline: 7.2256x; 7.2256x over previous
"""Causal BoW (running mean over T) Trainium2 kernel.

out[b, t, c] = sum_{s<=t} x[b, s, c] / (t+1)   for x of shape [32, 2048, 512] f32.

Sharding: batch B=32 across 8 NeuronCores (4 samples each), no cross-core comms.

Per-core algorithm (per sample [T=2048, C=512], 16 T-blocks of 128 rows):
  - f32 matmuls cost 4 cycles/row on the PE; float32r costs ~1 cycle/row but
    keeps only 11 mantissa bits. x is split on-chip into
    x_hi = round_f32r(x) (ACT copy) and x_lo = round_f32r(x - x_hi) (DVE sub);
    streaming both through the PE reconstructs full fp32 precision (verified
    bit-exact on HW) at ~2 cycles/row total.
  - Block scan: psum_j = U128^T.T @ xhi_j + U128^T.T @ xlo_j (U128 =
    upper-triangular ones). All scan matmuls share one weight matrix so the
    PE streams back-to-back (~240-330 ns per N=512 matmul).
  - Block offsets: accumulating matmuls with "step" selector weights
    (step_k[p, m] = 1 if m > k) produce off[m, c] = sum_{k<m} tot_k in one
    PSUM bank; split into off_hi/off_lo f32r rows.
  - Offset broadcast: off_hi[j] / off_lo[j] are scattered by two tiny
    SBUF->SBUF DMAs into partitions 0/1 of a per-sample staging tile (DMA
    moves data across partitions freely; compute engines cannot), then
    psum_j += ones2^T.T @ bo[:, j] — a K=2 matmul whose all-ones [2,128]
    weight is shared by every block, avoiding per-block weight reloads.
  - Eviction: Copy with per-partition scale recip[p, j] = 1/(j*128+p+1)
    applied while moving PSUM -> SBUF, alternating ACT/DVE.
  - All DMAs keep full 128-partition access patterns: odd partition counts
    (e.g. 127) defeat the HW-DGE multi-engine fanout and serialize all
    traffic onto one DMA engine (measured 7x regression).
"""

import numpy as np

import concourse.bass as bass
import concourse.bacc as bacc
import concourse.mybir as mybir
from concourse import tile
from concourse.bass_utils import run_bass_kernel_spmd

B, T, C = 32, 2048, 512
N_CORES = 8
BS = B // N_CORES          # samples per core
P = 128                    # partitions / T-block size
NBLK = T // P              # 16 blocks per sample
NH = NBLK // 2             # blocks per half-sample tile group
F32 = mybir.dt.float32
F32R = mybir.dt.float32r

_cache = {}


def _build():
    nc = bacc.Bacc()
    x = nc.dram_tensor("x", [BS, T, C], F32, kind="ExternalInput")
    u128 = nc.dram_tensor("u128", [P, P], F32R, kind="ExternalInput")
    stepm = nc.dram_tensor("stepm", [P, NBLK * NBLK], F32R, kind="ExternalInput")
    ones2 = nc.dram_tensor("ones2", [2, P], F32R, kind="ExternalInput")
    recip = nc.dram_tensor("recip", [P, NBLK], F32, kind="ExternalInput")
    y = nc.dram_tensor("y", [BS, T, C], F32, kind="ExternalOutput")

    HALF = NH * C

    with tile.TileContext(nc) as tc:
        with (
            tc.tile_pool(name="singles", bufs=1) as singles,
            tc.tile_pool(name="xp", bufs=2) as xpool,
            tc.tile_pool(name="xhp", bufs=2) as xhpool,
            tc.tile_pool(name="xlp", bufs=2) as xlpool,
            tc.tile_pool(name="op", bufs=2) as opool,
            tc.tile_pool(name="offp", bufs=2) as offpool,
            tc.tile_pool(name="bop", bufs=1) as bopool,
            tc.tile_pool(name="pblk", bufs=5, space="PSUM") as pblk,
            tc.tile_pool(name="poff", bufs=2, space="PSUM") as poff,
        ):
            u_t = singles.tile([P, P], F32R)
            nc.sync.dma_start(out=u_t[:], in_=u128[:])
            step_t = singles.tile([P, NBLK * NBLK], F32R)
            nc.sync.dma_start(out=step_t[:], in_=stepm[:])
            ones2_t = singles.tile([2, P], F32R)
            nc.sync.dma_start(out=ones2_t[:], in_=ones2[:])
            recip_t = singles.tile([P, NBLK], F32)
            nc.sync.dma_start(out=recip_t[:], in_=recip[:])

            for b in range(BS):
                xs = x[b].rearrange("(j p) c -> p j c", p=P)   # [128, 16, 512]
                ys = y[b].rearrange("(j p) c -> p j c", p=P)

                xhs, xls = [], []
                for h in range(2):
                    xt = xpool.tile([P, HALF], F32, tag="xt")
                    xt3 = xt.rearrange("p (j c) -> p j c", c=C)
                    nc.sync.dma_start(out=xt3[:], in_=xs[:, h * NH:(h + 1) * NH, :])
                    xh = xhpool.tile([P, HALF], F32R, tag="xh")
                    nc.scalar.copy(out=xh[:], in_=xt[:])
                    xl = xlpool.tile([P, HALF], F32R, tag="xl")
                    nc.vector.tensor_sub(out=xl[:], in0=xt[:], in1=xh[:].bitcast(F32))
                    xhs.append(xh)
                    xls.append(xl)

                # off[m, c] = sum_{k<m} (block-k column sum), one PSUM bank
                offp_t = poff.tile([NBLK, C], F32)
                for k in range(NBLK):
                    sel = step_t[:, k * NBLK:(k + 1) * NBLK]
                    for part, src in ((0, xhs), (1, xls)):
                        nc.tensor.matmul(
                            offp_t[:],
                            sel,
                            src[k // NH][:, (k % NH) * C:(k % NH + 1) * C],
                            start=(k == 0 and part == 0),
                            stop=(k == NBLK - 1 and part == 1),
                        )
                off_hi = offpool.tile([NBLK, C], F32R, tag="offhi")
                nc.scalar.copy(out=off_hi[:], in_=offp_t[:])
                off_lo = offpool.tile([NBLK, C], F32R, tag="offlo")
                nc.vector.tensor_sub(
                    out=off_lo[:], in0=offp_t[:], in1=off_hi[:].bitcast(F32)
                )

                # scatter offset rows to partitions 0/1 of the staging tile:
                # bo[0, j*C:(j+1)*C] = off_hi[j], bo[1, ...] = off_lo[j]
                bo = bopool.tile([2, NBLK * C], F32R)
                bo3 = bo.rearrange("p (j c) -> p j c", c=C)
                nc.sync.dma_start(out=bo3[0:1, :, :], in_=off_hi[:])
                nc.sync.dma_start(out=bo3[1:2, :, :], in_=off_lo[:])

                # main scan: every matmul's weights are either U or ones2
                for h in range(2):
                    ot = opool.tile([P, HALF], F32, tag="ot")
                    for jj in range(NH):
                        j = h * NH + jj
                        cs = slice(jj * C, (jj + 1) * C)
                        pb = pblk.tile([P, C], F32)
                        nc.tensor.matmul(pb[:], u_t[:], xhs[h][:, cs],
                                         start=True, stop=False)
                        nc.tensor.matmul(pb[:], u_t[:], xls[h][:, cs],
                                         start=False, stop=(j == 0))
                        if j > 0:
                            nc.tensor.matmul(
                                pb[:], ones2_t[:],
                                bo[:, j * C:(j + 1) * C],
                                start=False, stop=True,
                            )
                        if j % 2 == 0:
                            nc.scalar.mul(ot[:, cs], pb[:], recip_t[:, j:j + 1])
                        else:
                            nc.vector.tensor_scalar_mul(
                                ot[:, cs], pb[:], recip_t[:, j:j + 1]
                            )
                    ot3 = ot.rearrange("p (j c) -> p j c", c=C)
                    nc.sync.dma_start(
                        out=ys[:, h * NH:(h + 1) * NH, :], in_=ot3[:]
                    )
    nc.finalize()
    return nc


def _consts():
    u = np.triu(np.ones((P, P), dtype=np.float32))
    step = np.zeros((P, NBLK * NBLK), dtype=np.float32)
    for k in range(NBLK):
        for m in range(NBLK):
            if m > k:
                step[:, k * NBLK + m] = 1.0
    ones2 = np.ones((2, P), dtype=np.float32)
    recip = (1.0 / np.arange(1, T + 1, dtype=np.float32)).reshape(NBLK, P).T.copy()
    return u, step, ones2, recip


def run(x, trace=False):
    x = np.ascontiguousarray(np.asarray(x, dtype=np.float32))
    assert x.shape == (B, T, C), x.shape
    if "nc" not in _cache:
        _cache["nc"] = _build()
    nc = _cache["nc"]
    u, step, ones2, recip = _consts()
    in_maps = [
        {
            "x": np.ascontiguousarray(x[i * BS:(i + 1) * BS]),
            "u128": u,
            "stepm": step,
            "ones2": ones2,
            "recip": recip,
        }
        for i in range(N_CORES)
    ]
    res = run_bass_kernel_spmd(nc, in_maps, list(range(N_CORES)), trace=trace)
    y = np.concatenate([res.results[i]["y"] for i in range(N_CORES)], axis=0)
    return y, res.exec_time_ns


def kernel(x):
    y, _ = run(x, trace=False)
    return y


# revision 18
# speedup vs baseline: 8.2176x; 1.1373x over previous
"""Causal BoW (running mean over T) Trainium2 kernel.

out[b, t, c] = sum_{s<=t} x[b, s, c] / (t+1)   for x of shape [32, 2048, 512] f32.

Sharding: batch B=32 across 8 NeuronCores (4 samples each), no cross-core comms.

Per-core algorithm (per sample [T=2048, C=512], 16 T-blocks of 128 rows):
  - f32 matmuls cost 4 cycles/row on the PE; float32r costs ~1 cycle/row but
    keeps only 11 mantissa bits. x is split on-chip into
    x_hi = round_f32r(x) (ACT copy) and x_lo = round_f32r(x - x_hi) (DVE sub);
    streaming both through the PE reconstructs full fp32 precision (verified
    bit-exact on HW) at ~2 cycles/row total.
  - Block scan: psum_j = U128^T.T @ xhi_j + U128^T.T @ xlo_j (U128 =
    upper-triangular ones). All scan matmuls share one weight matrix so the
    PE streams back-to-back (~240-330 ns per N=512 matmul).
  - Block offsets: accumulating matmuls with "step" selector weights
    (step_k[p, m] = 1 if m > k) produce off[m, c] = sum_{k<m} tot_k in one
    PSUM bank; split into off_hi/off_lo f32r rows.
  - Offset broadcast: off_hi[j] / off_lo[j] are scattered by two tiny
    SBUF->SBUF DMAs into partitions 0/1 of a per-sample staging tile (DMA
    moves data across partitions freely; compute engines cannot), then
    psum_j += ones2^T.T @ bo[:, j] — a K=2 matmul whose all-ones [2,128]
    weight is shared by every block, avoiding per-block weight reloads.
  - Eviction: Copy with per-partition scale recip[p, j] = 1/(j*128+p+1)
    applied while moving PSUM -> SBUF, alternating ACT/DVE.
  - All DMAs keep full 128-partition access patterns: odd partition counts
    (e.g. 127) defeat the HW-DGE multi-engine fanout and serialize all
    traffic onto one DMA engine (measured 7x regression).
"""

import numpy as np

import concourse.bass as bass
import concourse.bacc as bacc
import concourse.mybir as mybir
from concourse import tile
from concourse.bass_utils import run_bass_kernel_spmd

B, T, C = 32, 2048, 512
N_CORES = 8
BS = B // N_CORES          # samples per core
P = 128                    # partitions / T-block size
NBLK = T // P              # 16 blocks per sample
NQ = 4                     # tile groups per sample
NH = NBLK // NQ            # blocks per tile group (4)
F32 = mybir.dt.float32
F32R = mybir.dt.float32r

_cache = {}


def _build():
    nc = bacc.Bacc()
    x = nc.dram_tensor("x", [BS, T, C], F32, kind="ExternalInput")
    u128 = nc.dram_tensor("u128", [P, P], F32R, kind="ExternalInput")
    stepm = nc.dram_tensor("stepm", [P, NBLK * NBLK], F32R, kind="ExternalInput")
    ones2 = nc.dram_tensor("ones2", [2, P], F32R, kind="ExternalInput")
    recip = nc.dram_tensor("recip", [P, NBLK], F32, kind="ExternalInput")
    y = nc.dram_tensor("y", [BS, T, C], F32, kind="ExternalOutput")

    HALF = NH * C

    with tile.TileContext(nc) as tc:
        with (
            tc.tile_pool(name="singles", bufs=1) as singles,
            tc.tile_pool(name="xp", bufs=3) as xpool,
            tc.tile_pool(name="xhp", bufs=5) as xhpool,
            tc.tile_pool(name="xlp", bufs=5) as xlpool,
            tc.tile_pool(name="op", bufs=4) as opool,
            tc.tile_pool(name="offp", bufs=2) as offpool,
            tc.tile_pool(name="bop", bufs=1) as bopool,
            tc.tile_pool(name="pblk", bufs=6, space="PSUM") as pblk,
            tc.tile_pool(name="poff", bufs=2, space="PSUM") as poff,
        ):
            u_t = singles.tile([P, P], F32R)
            nc.sync.dma_start(out=u_t[:], in_=u128[:])
            step_t = singles.tile([P, NBLK * NBLK], F32R)
            nc.sync.dma_start(out=step_t[:], in_=stepm[:])
            ones2_t = singles.tile([2, P], F32R)
            nc.sync.dma_start(out=ones2_t[:], in_=ones2[:])
            recip_t = singles.tile([P, NBLK], F32)
            nc.sync.dma_start(out=recip_t[:], in_=recip[:])

            for b in range(BS):
                xs = x[b].rearrange("(j p) c -> p j c", p=P)   # [128, 16, 512]
                ys = y[b].rearrange("(j p) c -> p j c", p=P)

                xhs, xls = [], []
                for h in range(NQ):
                    xt = xpool.tile([P, HALF], F32, tag="xt")
                    xt3 = xt.rearrange("p (j c) -> p j c", c=C)
                    nc.sync.dma_start(out=xt3[:], in_=xs[:, h * NH:(h + 1) * NH, :])
                    xh = xhpool.tile([P, HALF], F32R, tag="xh")
                    nc.scalar.copy(out=xh[:], in_=xt[:])
                    xl = xlpool.tile([P, HALF], F32R, tag="xl")
                    nc.vector.tensor_sub(out=xl[:], in0=xt[:], in1=xh[:].bitcast(F32))
                    xhs.append(xh)
                    xls.append(xl)

                # off[m, c] = sum_{k<m} (block-k column sum), one PSUM bank
                offp_t = poff.tile([NBLK, C], F32)
                for k in range(NBLK):
                    sel = step_t[:, k * NBLK:(k + 1) * NBLK]
                    for part, src in ((0, xhs), (1, xls)):
                        nc.tensor.matmul(
                            offp_t[:],
                            sel,
                            src[k // NH][:, (k % NH) * C:(k % NH + 1) * C],
                            start=(k == 0 and part == 0),
                            stop=(k == NBLK - 1 and part == 1),
                        )
                off_hi = offpool.tile([NBLK, C], F32R, tag="offhi")
                nc.scalar.copy(out=off_hi[:], in_=offp_t[:])
                off_lo = offpool.tile([NBLK, C], F32R, tag="offlo")
                nc.vector.tensor_sub(
                    out=off_lo[:], in0=offp_t[:], in1=off_hi[:].bitcast(F32)
                )

                # scatter offset rows to partitions 0/1 of the staging tile:
                # bo[0, j*C:(j+1)*C] = off_hi[j], bo[1, ...] = off_lo[j]
                bo = bopool.tile([2, NBLK * C], F32R)
                bo3 = bo.rearrange("p (j c) -> p j c", c=C)
                nc.sync.dma_start(out=bo3[0:1, :, :], in_=off_hi[:])
                nc.sync.dma_start(out=bo3[1:2, :, :], in_=off_lo[:])

                # main scan: every matmul's weights are either U or ones2;
                # evictions all on DVE (ACT reads PSUM at ~half DVE's rate,
                # stretching the window in which PE matmuls contend with
                # eviction reads for PSUM bandwidth)
                for h in range(NQ):
                    ot = opool.tile([P, HALF], F32, tag="ot")
                    for jj in range(NH):
                        j = h * NH + jj
                        cs = slice(jj * C, (jj + 1) * C)
                        pb = pblk.tile([P, C], F32)
                        nc.tensor.matmul(pb[:], u_t[:], xhs[h][:, cs],
                                         start=True, stop=False)
                        nc.tensor.matmul(pb[:], u_t[:], xls[h][:, cs],
                                         start=False, stop=(j == 0))
                        if j > 0:
                            nc.tensor.matmul(
                                pb[:], ones2_t[:],
                                bo[:, j * C:(j + 1) * C],
                                start=False, stop=True,
                            )
                        nc.vector.tensor_scalar_mul(
                            ot[:, cs], pb[:], recip_t[:, j:j + 1]
                        )
                    ot3 = ot.rearrange("p (j c) -> p j c", c=C)
                    nc.sync.dma_start(
                        out=ys[:, h * NH:(h + 1) * NH, :], in_=ot3[:]
                    )
    nc.finalize()
    return nc


def _consts():
    u = np.triu(np.ones((P, P), dtype=np.float32))
    step = np.zeros((P, NBLK * NBLK), dtype=np.float32)
    for k in range(NBLK):
        for m in range(NBLK):
            if m > k:
                step[:, k * NBLK + m] = 1.0
    ones2 = np.ones((2, P), dtype=np.float32)
    recip = (1.0 / np.arange(1, T + 1, dtype=np.float32)).reshape(NBLK, P).T.copy()
    return u, step, ones2, recip


def run(x, trace=False):
    x = np.ascontiguousarray(np.asarray(x, dtype=np.float32))
    assert x.shape == (B, T, C), x.shape
    if "nc" not in _cache:
        _cache["nc"] = _build()
    nc = _cache["nc"]
    u, step, ones2, recip = _consts()
    in_maps = [
        {
            "x": np.ascontiguousarray(x[i * BS:(i + 1) * BS]),
            "u128": u,
            "stepm": step,
            "ones2": ones2,
            "recip": recip,
        }
        for i in range(N_CORES)
    ]
    res = run_bass_kernel_spmd(nc, in_maps, list(range(N_CORES)), trace=trace)
    y = np.concatenate([res.results[i]["y"] for i in range(N_CORES)], axis=0)
    return y, res.exec_time_ns


def kernel(x):
    y, _ = run(x, trace=False)
    return y
